# revision 37
# baseline (speedup 1.0000x reference)
"""Multi-head attention kernel for 8 TRN2 NeuronCores.

Problem: x[4,2048,1024] -> qkv proj (w_qkv[1024,3072]) -> 16-head attention
(dim_head=64, scale=1024**-0.5) -> out proj (w_out[1024,1024] + b_out).

Sharding: core c in 0..7 handles batch b=c//2, head-group g=c%2 (8 heads).
Each core computes a partial output y_partial = attn_out_g @ w_out[rows_g];
host sums the pair (the tensor-parallel all-reduce, done at unshard time).

Layout strategy (zero on-chip transposes):
  - host supplies xT = x[b].T fp16, plus fp8e4 DoubleRow-packed copies
    xT8 / wqk8 (w_qk pre-scaled x64; exp scale absorbs the 1/4096)
  - qkT = (x @ w_qk).T via fp8 DoubleRow matmuls (K=256/matmul, 0.5
    cycles/row) -> [c, i] fp16
  - V   = x @ w_v fp16 (V must stay high precision: its error lands
    directly in the output)
  - S^T = k_h @ q_h^T per head:            lhsT=kT slice, rhs=qT slice
          -> [keys, q]; head pairs in PE row-groups 0-63 / 64-127
  - P   = exp(S^T * lam)  (no max subtraction: |arg| < ~1)
  - exp engine split: 12 of 16 key-chunks per pass on ScalarE (fp16 pt,
    fp16 OT matmuls); kc pairs (4,5),(8,9) offloaded: DVE computes
    u = lam*S (frees the stq psum fast), then DVE or GpSimd compute the
    quadratic expm1 residual R = u*(1 + u/2) straight into fp8e4
    ([128,2,1024] kc-pair tiles). |u| <~ 0.7 so the cubic error is
    ~0.1%, and quantizing R (not P~1) keeps fp8 noise ~0.4%.
  - offloaded pairs' O^T accumulate via fp8 DoubleRow matmuls
    (lhsT = [v|1] fp8 pair tiles, rhs = R tiles); the missing sum_j v_j
    (from P = 1 + R) is a per-partition constant folded into the
    psum->sbuf raw copy (tensor_scalar add of a precomputed vsum AP,
    computed once in the prelude via ones-vector matmuls; v_full fp16
    is used there so v's fp8 error only enters through R*(v8-v16)).
  - normalize off critical path: denominator rows DMA-gathered
    partition-major into [128,8] so the DVE iterative divide is
    free-dim-bound (~0.2us, not 3.3us); 1/s bounced through DRAM and
    broadcast-DMA'd across partitions
  - y = sum_h (O_h^T).T @ w_out_h fp16
Pipeline: stq psum triple-buffered ("stq" tag also hosts prelude/fill/
out-proj chains); OT matmuls trail two kc behind; offloaded-pair
DoubleRow OTs are emitted only when their fp8 tiles are long ready
(pair 0 at kc11, pair 1 in the pass-end drain) so the strict-FIFO PE
never blocks on the elementwise engines; TWO score matmuls + ACTs are
hoisted across each pass boundary to cover the OT/DR drain; remaining
qkT fill chains run in the ot-bank window at pass boundaries.
Engine budget per pass (us): ScalarE ~13.3 exp (the pacer), DVE ~11
(u-copies, quadratic, raws+vsum, recip, fill copies), GpSimd ~10
(quadratic for one kc/pair, normalize muls), PE ~12.5.
Run-to-run variance: back-to-back runs heat the device and inflate all
engine clocks ~20% (363us cool, ~430us hot).
"""

import numpy as np

B, N, D = 4, 2048, 1024
HEADS, DH = 16, 64
HP = HEADS // 2          # heads per core
GDIM = HP * DH           # 512 columns per head-group
SCALE = float(D) ** -0.5
NCORES = 8

_CACHE = {}


def _build():
    from contextlib import ExitStack

    import concourse.bass as bass
    import concourse.tile as tile
    from concourse import bacc, mybir

    F16 = mybir.dt.float16
    F32 = mybir.dt.float32
    F8 = mybir.dt.float8e4
    DR = mybir.MatmulPerfMode.DoubleRow
    EXP = mybir.ActivationFunctionType.Exp
    LN = mybir.ActivationFunctionType.Ln

    nc = bacc.Bacc(None, target_bir_lowering=False)

    xT_d = nc.declare_dram_parameter("xT", [D, N], F16, isOutput=False)
    xT8_d = nc.declare_dram_parameter("xT8", [4, 128, 2 * N], F8, isOutput=False)
    wqk8_d = nc.declare_dram_parameter("wqk8", [4, 128, 2 * 2 * GDIM], F8,
                                       isOutput=False)
    wv_d = nc.declare_dram_parameter("wv", [D, GDIM], F16, isOutput=False)
    wo_d = nc.declare_dram_parameter("wo", [4, 128, D], F16, isOutput=False)
    bias_d = nc.declare_dram_parameter("bias", [D], F32, isOutput=False)
    out_d = nc.declare_dram_parameter("out", [N, D], F32, isOutput=True)

    with tile.TileContext(nc) as tc, ExitStack() as ctx:
        persist = ctx.enter_context(tc.tile_pool(name="persist", bufs=1))
        ptp = ctx.enter_context(tc.tile_pool(name="ptp", bufs=5))
        rawp = ctx.enter_context(tc.tile_pool(name="rawp", bufs=5))
        tiny = ctx.enter_context(tc.tile_pool(name="tiny", bufs=4))
        ypool = ctx.enter_context(tc.tile_pool(name="ypool", bufs=2))
        dramp = ctx.enter_context(tc.tile_pool(name="dramp", bufs=4,
                                               space="DRAM"))
        # PSUM budget (8 banks): stq [128,1024] x2 bufs = 4, ot x4 = 4... see
        # tags: "stq" 2-bank tiles bufs=2, "ot0..3" 1 bank each, "qf" 1 bank
        mm = ctx.enter_context(tc.tile_pool(name="mm", bufs=3, space="PSUM"))
        acc = ctx.enter_context(tc.tile_pool(name="acc", bufs=1, space="PSUM"))

        # ---- persistent SBUF tiles -------------------------------------
        xT = [persist.tile([128, N], F16, name=f"xT{e}", tag=f"xT{e}")
              for e in range(8)]
        xT8 = [persist.tile([128, 2, N], F8, name=f"xT8_{p}", tag=f"xT8_{p}")
               for p in range(4)]
        wqk8 = [persist.tile([128, 2, 2 * GDIM], F8, name=f"wqk8_{p}",
                             tag=f"wqk8_{p}") for p in range(4)]
        wv = [persist.tile([128, GDIM], F16, name=f"wv{e}", tag=f"wv{e}")
              for e in range(8)]
        wo = [persist.tile([128, D], F16, name=f"wo{tp}", tag=f"wo{tp}")
              for tp in range(4)]
        bias = persist.tile([128, D], F32, tag="bias")
        qkT = [persist.tile([128, N], F16, name=f"qkT{c}", tag=f"qkT{c}")
               for c in range(8)]
        vt = [persist.tile([128, HP, DH + 1], F16, name=f"v{kc}", tag=f"v{kc}")
              for kc in range(16)]
        otn = [persist.tile([128, N], F16, name=f"otn{tp}", tag=f"otn{tp}")
               for tp in range(4)]
        # fp8 V pair-tiles + per-head-pair vsum corrections for the
        # DVE/Pool-offloaded kc pairs (DoubleRow P-residual attention)
        OFF_PAIRS = ((6, 7), (10, 11))   # kc pairs offloaded per pass
        OFF = {6: (0, 0), 7: (0, 1), 10: (1, 0), 11: (1, 1)}
        VP = DH + 4              # pad head stride to 4B alignment for LDW
        vt8 = [persist.tile([128, 2, HP, VP], F8, name=f"vt8_{i}",
                            tag=f"vt8_{i}") for i in range(2)]
        vsv = [persist.tile([65, 2], F32, name=f"vsv{t}", tag=f"vsv{t}")
               for t in range(4)]
        ones16 = persist.tile([128, 1], F16, tag="ones16")
        pones = persist.tile([2, 512], F32, tag="pones")
        upool = ctx.enter_context(tc.tile_pool(name="upool", bufs=2))
        hpool = ctx.enter_context(tc.tile_pool(name="hpool", bufs=2))
        pt8p = ctx.enter_context(tc.tile_pool(name="pt8p", bufs=2))
        LAM = SCALE / 4096.0

        dmaq = [nc.sync, nc.scalar]
        for e in range(8):
            dmaq[e % 2].dma_start(out=xT[e],
                                  in_=xT_d[e * 128:(e + 1) * 128, :])
            dmaq[(e + 1) % 2].dma_start(out=wv[e],
                                        in_=wv_d[e * 128:(e + 1) * 128, :])
        for p in range(4):
            dmaq[p % 2].dma_start(
                out=xT8[p],
                in_=xT8_d[p].rearrange("k (two n) -> k two n", two=2))
            dmaq[(p + 1) % 2].dma_start(
                out=wqk8[p],
                in_=wqk8_d[p].rearrange("k (two m) -> k two m", two=2))
        for tp in range(4):
            dmaq[tp % 2].dma_start(out=wo[tp], in_=wo_d[tp])
        bias_ap = bias_d[:]
        nc.sync.dma_start(
            out=bias,
            in_=bass.AP(tensor=bias_ap.tensor, offset=bias_ap.offset,
                        ap=[[0, 128]] + list(bias_ap.ap)),
        )
        for kc in range(16):
            nc.vector.memset(vt[kc][:, :, DH:DH + 1], 1.0)

        # ---- PE warm-up: dummy matmuls during the input-DMA window ------
        wu = persist.tile([128, 512], F16, tag="wu")
        nc.vector.memset(wu, 0.0)
        wps = mm.tile([128, 1024], F32, name="stq", tag="stq")
        for r in range(32):
            nc.tensor.matmul(wps[:, 0:512], lhsT=wu[:, 0:128], rhs=wu,
                             start=True, stop=True)

        # ---- prelude: V first, then qkT ordered so pair-0 chunks land
        # last (dense PE hand-off into the first attention pass).
        # Chains round-robin over all 6 psum slots (2 stq + 4 ot tags) so
        # the psum->sbuf copies never stall the matmul stream.
        PSLOTS = [(acc, "ot0"), (acc, "ot1"), (mm, "stq"), (mm, "stq")]

        def v_chain_small(it, slot):
            pool_, tag_ = PSLOTS[slot % 4]
            ps = pool_.tile([128, 512], F32, name=f"pv{it}", tag=tag_)
            for e in range(8):
                yield nc.tensor.matmul(
                    ps, lhsT=xT[e][:, it * 128:(it + 1) * 128],
                    rhs=wv[e], start=(e == 0), stop=(e == 7))
            yield nc.vector.tensor_copy(
                vt[it][:, :, 0:DH],
                ps.rearrange("p (h d) -> p h d", h=HP))

        def qkv_chain_small(c, iq, slot):
            pool_, tag_ = PSLOTS[slot % 4]
            ps = pool_.tile([128, 512], F32, name=f"pq{c}_{iq}", tag=tag_)
            for p in range(4):
                yield nc.tensor.matmul(
                    ps, lhsT=wqk8[p][:, :, c * 128:(c + 1) * 128],
                    rhs=xT8[p][:, :, iq * 512:(iq + 1) * 512],
                    start=(p == 0), stop=(p == 3), perf_mode=DR)
            yield nc.vector.tensor_copy(
                qkT[c][:, iq * 512:(iq + 1) * 512], ps)

        gens = []
        for it in range(16):
            gens.append(("v", it))
        for c in (0, 4):
            for iq in range(4):
                gens.append(("qk", c, iq))
        # interleave: 2 big stq chains run as before; others on ot slots
        active = []
        gi = 0
        slot_rr = 0
        streams = []
        for g in gens:
            if g[0] == "v":
                streams.append(v_chain_small(g[1], slot_rr % 4))
            else:
                streams.append(qkv_chain_small(g[1], g[2], slot_rr % 4))
            slot_rr += 1
        # emit round-robin across 6 concurrent streams
        live = streams[:6]
        nxt = 6
        while live:
            done = []
            for s in live:
                if next(s, None) is None:
                    done.append(s)
            for s in done:
                live.remove(s)
                if nxt < len(streams):
                    live.append(streams[nxt])
                    nxt += 1

        # ---- offload prep: fp8 V pair-tiles, vsum corrections ----------
        nc.vector.memset(ones16, 1.0)
        nc.vector.memset(pones, 1.0)
        for i, (kA, kB) in enumerate(OFF_PAIRS):
            nc.vector.memset(vt8[i][:, :, :, DH:DH + 1], 1.0)
            nc.vector.memset(vt8[i][:, :, :, DH + 1:VP], 0.0)
            for j, kc in enumerate((kA, kB)):
                with nc.allow_low_precision(reason="v residual term in fp8"):
                    nc.vector.tensor_copy(vt8[i][:, j, :, 0:DH],
                                          vt[kc][:, :, 0:DH])
        # vsum[d] = sum_k v_full[k, h, d] over the offloaded kc chunks;
        # lands as raw-copy bias so the DR matmuls only carry P-1 residuals
        vs_ps = acc.tile([1, 512], F32, name="vsps", tag="ot0")
        vkcs = [kc for p_ in OFF_PAIRS for kc in p_]
        for i, kc in enumerate(vkcs):
            nc.tensor.matmul(vs_ps, lhsT=ones16, rhs=vt[kc][:, :, 0:DH],
                             start=(i == 0), stop=(i == len(vkcs) - 1))
        vs_sb = tiny.tile([1, 512], F32, name="vssb", tag="vssb", bufs=1)
        nc.vector.tensor_copy(vs_sb, vs_ps)
        dvs = dramp.tile([512], F32, name="dvs", tag="dvs")
        nc.sync.dma_start(out=dvs, in_=vs_sb)
        for t in range(4):
            nc.vector.memset(vsv[t][64:65, :], float(len(vkcs) * 128))
            for j in (0, 1):
                nc.sync.dma_start(out=vsv[t][0:64, j:j + 1],
                                  in_=dvs[(2 * t + j) * 64:(2 * t + j + 1) * 64])

        # ---- attention: head pairs x q-quarters. Each stq tile holds both
        # heads' scores side by side ([A 512 | B 512]) so one FD=1024 exp
        # covers the pair, and each pass needs only TWO ot banks. The two
        # freed PSUM banks (qf0/qf1) host interleaved qkv filler chains
        # that hide most of the old prelude under the exp stream. ----------
        def emit_st_exp(t, qc, kc):
            qch, kch = t, 4 + t
            stq = mm.tile([128, 1024], F32, name="stq", tag="stq")
            nc.tensor.matmul(
                stq[:, 0:512],
                lhsT=qkT[kch][0:64, kc * 128:(kc + 1) * 128],
                rhs=qkT[qch][0:64, qc * 512:(qc + 1) * 512],
                start=True, stop=True)
            nc.tensor.matmul(
                stq[:, 512:1024],
                lhsT=qkT[kch][64:128, kc * 128:(kc + 1) * 128],
                rhs=qkT[qch][64:128, qc * 512:(qc + 1) * 512],
                start=True, stop=True)
            pt = ptp.tile([128, 1024], F16, name="pt", tag="pt")
            nc.scalar.activation(pt, stq, EXP, scale=SCALE / 4096.0)
            return pt

        # filler state: remaining qkT chunks as 9-item chains (8 MM + copy)
        fill_specs = []
        for tt in range(1, 4):
            for c in (tt, 4 + tt):
                for iq in range(4):
                    fill_specs.append((c, iq))

        def fill_chain(c, iq, slot):
            """Whole qkT fill chain in the ot-bank pass-boundary window."""
            ps = acc.tile([128, 512], F32, name=f"qf{c}_{iq}",
                          tag=f"ot{slot}")
            for p in range(4):
                nc.tensor.matmul(
                    ps, lhsT=wqk8[p][:, :, c * 128:(c + 1) * 128],
                    rhs=xT8[p][:, :, iq * 512:(iq + 1) * 512],
                    start=(p == 0), stop=(p == 3), perf_mode=DR)
            nc.vector.tensor_copy(
                qkT[c][:, iq * 512:(iq + 1) * 512], ps)

        fill_state = {"idx": 0}

        def emit_fill(n):
            for k in range(n):
                if fill_state["idx"] >= len(fill_specs):
                    return
                c, iq = fill_specs[fill_state["idx"]]
                fill_state["idx"] += 1
                fill_chain(c, iq, k % 2)

        def emit_st(t, qc, kc, stq):
            qch, kch = t, 4 + t
            nc.tensor.matmul(
                stq[:, 0:512],
                lhsT=qkT[kch][0:64, kc * 128:(kc + 1) * 128],
                rhs=qkT[qch][0:64, qc * 512:(qc + 1) * 512],
                start=True, stop=True)
            nc.tensor.matmul(
                stq[:, 512:1024],
                lhsT=qkT[kch][64:128, kc * 128:(kc + 1) * 128],
                rhs=qkT[qch][64:128, qc * 512:(qc + 1) * 512],
                start=True, stop=True)

        passes = [(t, qc) for t in range(4) for qc in range(4)]
        hoisted = None
        for pi, (t, qc) in enumerate(passes):
            hA, hB = 2 * t, 2 * t + 1
            otA = acc.tile([65, 512], F32, name=f"otA{pi}", tag="ot0")
            otB = acc.tile([65, 512], F32, name=f"otB{pi}", tag="ot1")

            def emit_ot(entry):
                _, kc, pt = entry
                st = (kc == 0)
                nc.tensor.matmul(otA, lhsT=vt[kc][:, hA, :],
                                 rhs=pt[:, 0:512], start=st, stop=False,
                                 skip_group_check=True)
                nc.tensor.matmul(otB, lhsT=vt[kc][:, hB, :],
                                 rhs=pt[:, 512:1024], start=st, stop=False,
                                 skip_group_check=True)

            def emit_ot_dr(pid, pt8, sp):
                nc.tensor.matmul(otA, lhsT=vt8[pid][:, :, hA, 0:DH + 1],
                                 rhs=pt8[:, :, 0:512], start=False,
                                 stop=sp, perf_mode=DR,
                                 skip_group_check=True)
                nc.tensor.matmul(otB, lhsT=vt8[pid][:, :, hB, 0:DH + 1],
                                 rhs=pt8[:, :, 512:1024], start=False,
                                 stop=sp, perf_mode=DR,
                                 skip_group_check=True)

            pt_hist = []
            if hoisted is not None:
                for hk, hpt in hoisted:
                    pt_hist.append(("f16", hk, hpt))
                kc_start = len(hoisted)
                hoisted = None
            else:
                kc_start = 0
            cur_pt8 = {}
            dr_entries = []
            for kc in range(kc_start, 16):
                if pi == 0 and kc in (0, 1, 2):
                    for dj in (0, 1):
                        nc.tensor.matmul([otA, otB][dj], lhsT=wu[:, 0:65],
                                         rhs=wu, start=True, stop=True,
                                         skip_group_check=True)
                if kc in OFF:
                    pid, j = OFF[kc]
                    stq = mm.tile([128, 1024], F32, name="stq", tag="stq")
                    emit_st(t, qc, kc, stq)
                    u = upool.tile([128, 1024], F16, name="u", tag="u")
                    nc.vector.tensor_scalar_mul(u, stq, LAM)
                    # quadratic expm1: R = u*(1 + u/2); alternate the
                    # second stage between DVE and GpSimd to split load
                    eng = nc.gpsimd if j == 0 else nc.vector
                    h = hpool.tile([128, 1024], F16, name="h", tag="h")
                    eng.tensor_scalar(h, u, 0.5, 1.0,
                                      mybir.AluOpType.mult,
                                      mybir.AluOpType.add)
                    if j == 0:
                        cur_pt8[pid] = pt8p.tile([128, 2, 1024], F8,
                                                 name="pt8", tag="pt8")
                    with nc.allow_low_precision(reason="P-1 residual fp8"):
                        eng.tensor_tensor(cur_pt8[pid][:, j, :], u, h,
                                          mybir.AluOpType.mult)
                    if j == 1:
                        dr_entries.append((pid, cur_pt8[pid]))
                else:
                    pt = emit_st_exp(t, qc, kc)
                    pt_hist.append(("f16", kc, pt))
                if len(pt_hist) > 2:
                    emit_ot(pt_hist.pop(0))
                if kc == 13 and dr_entries:
                    # pair-0 residual matmuls: fp8 tiles ready since ~kc11
                    pid0, pt80 = dr_entries.pop(0)
                    emit_ot_dr(pid0, pt80, sp=False)
            if pi + 1 < len(passes):
                nt, nqc = passes[pi + 1]
                hoisted = [(0, emit_st_exp(nt, nqc, 0)),
                           (1, emit_st_exp(nt, nqc, 1))]
            for entry in pt_hist:
                emit_ot(entry)
            for di, (pid, pt8) in enumerate(dr_entries):
                emit_ot_dr(pid, pt8, sp=(di == len(dr_entries) - 1))

            # normalize the two heads (off critical path); raw copy folds
            # in the offloaded chunks' sum_v correction + denominator count
            last_pass = (pi == len(passes) - 1)
            raws, rcs, bcs = {}, {}, {}
            for j, (ott, hh) in enumerate(((otA, hA), (otB, hB))):
                raw = rawp.tile([65, 512], F16, name="raw", tag="raw")
                nc.vector.tensor_scalar(raw, ott, vsv[t][:, j:j + 1], None,
                                        mybir.AluOpType.add)
                raws[j] = raw
            if last_pass:
                for j in (0, 1):
                    rc = tiny.tile([65, 512], F32, name="rc", tag="rc",
                                   bufs=2)
                    lntmp = tiny.tile([65, 512], F32, name="lntmp",
                                      tag="lntmp", bufs=2)
                    nc.scalar.activation(lntmp[64:65, :], raws[j][64:65, :],
                                         LN)
                    nc.scalar.activation(rc[64:65, :], lntmp[64:65, :],
                                         EXP, scale=-1.0)
                    rcs[j] = (rc, 64)
            else:
                # gather denominators partition-major ([128,8]: q%...=row)
                # so the DVE iterative divide sees free-size 8, not 512
                den2 = tiny.tile([128, 8], F16, name="den2", tag="den2",
                                 bufs=2)
                for j in (0, 1):
                    nc.sync.dma_start(out=den2[:, j * 4:(j + 1) * 4],
                                      in_=raws[j][64:65, :])
                rcb = tiny.tile([128, 8], F32, name="rcb", tag="rcb", bufs=2)
                with nc.allow_low_precision(reason="1/s fits f16"):
                    nc.vector.reciprocal(rcb, den2)
                rcs = {0: (rcb, 0), 1: (rcb, 1)}
            shifted = {}
            for j in (0, 1):
                odd = (j == 1)
                rt, rr = rcs[j]
                dsc = dramp.tile([512], F32, name="dsc", tag="dsc")
                if last_pass:
                    nc.sync.dma_start(out=dsc, in_=rt[rr:rr + 1, :])
                else:
                    nc.sync.dma_start(out=dsc, in_=rt[:, j * 4:(j + 1) * 4])
                bc = tiny.tile([128, 512], F32, name="bc", tag="bc", bufs=2)
                dap = dsc[:]
                po = 64 if odd else 0
                nc.sync.dma_start(
                    out=bc[po:po + 64, :],
                    in_=bass.AP(tensor=dap.tensor, offset=dap.offset,
                                ap=[[0, 64]] + list(dap.ap)))
                bcs[j] = bc
                if odd:
                    rdsc = dramp.tile([64, 512], F16, name="rdsc",
                                      tag="rdsc", bufs=2)
                    nc.sync.dma_start(out=rdsc, in_=raws[j][0:64, :])
                    sh = rawp.tile([128, 512], F16, name="sh", tag="sh",
                                   bufs=2)
                    nc.sync.dma_start(out=sh[64:128, :], in_=rdsc[:])
                    shifted[j] = sh
            nc.gpsimd.tensor_tensor(
                otn[t][0:64, qc * 512:(qc + 1) * 512],
                raws[0][0:64, :], bcs[0][0:64, :], mybir.AluOpType.mult)
            nc.gpsimd.tensor_tensor(
                otn[t][64:128, qc * 512:(qc + 1) * 512],
                shifted[1][64:128, :], bcs[1][64:128, :],
                mybir.AluOpType.mult)
            if t < 3:
                emit_fill(2)

        # ---- output projection: 32 narrow chains, 6 psum slots ---------
        ptags = ["stq", "stq", "stq", "ot0", "ot1"]
        ppools = [mm, mm, mm, acc, acc]
        ci = 0
        for it in range(16):
            for half in range(2):
                tag = ptags[ci % 5]
                ps = ppools[ci % 5].tile([128, 512], F32,
                                         name=f"pj{ci}", tag=tag)
                ci += 1
                e0 = half * 512
                for tp in range(4):
                    nc.tensor.matmul(
                        ps, lhsT=otn[tp][:, it * 128:(it + 1) * 128],
                        rhs=wo[tp][:, e0:e0 + 512],
                        start=(tp == 0), stop=(tp == 3))
                yt = ypool.tile([128, 512], F32, name="yt", tag="yt",
                                bufs=4)
                nc.vector.tensor_add(yt, ps, bias[:, e0:e0 + 512])
                yq = nc.sync if ci % 2 else nc.scalar
                yq.dma_start(
                    out=out_d[it * 128:(it + 1) * 128, e0:e0 + 512], in_=yt)

    nc.compile()
    return nc


def _in_maps(x, w_qkv, w_out, b_out):
    import ml_dtypes

    F8NP = ml_dtypes.float8_e4m3
    x = np.asarray(x, dtype=np.float32)
    w_qkv = np.asarray(w_qkv, dtype=np.float32)
    w_out = np.asarray(w_out, dtype=np.float32)
    b_out = np.asarray(b_out, dtype=np.float32)

    def dr_pack(m):
        # [1024 rows, C] -> [4, 128, 2*C]: DoubleRow k-tile pairs packed on
        # the free axis: tile p holds rows 256p+part (j=0) / 256p+128+part
        # (j=1)
        c = m.shape[1]
        return np.ascontiguousarray(
            m.reshape(4, 2, 128, c).transpose(0, 2, 1, 3).reshape(
                4, 128, 2 * c))

    maps = []
    xt8 = {}
    for b in range(B):
        xt8[b] = dr_pack(np.ascontiguousarray(x[b].T)).astype(F8NP)
    for c in range(NCORES):
        b, g = c // 2, c % 2
        qcols = w_qkv[:, g * GDIM:(g + 1) * GDIM]
        kcols = w_qkv[:, D + g * GDIM:D + (g + 1) * GDIM]
        vcols = w_qkv[:, 2 * D + g * GDIM:2 * D + (g + 1) * GDIM]
        wqk64 = np.concatenate([qcols, kcols], axis=1) * 64.0
        maps.append({
            "xT": np.ascontiguousarray(x[b].T).astype(np.float16),
            "xT8": xt8[b],
            "wqk8": dr_pack(wqk64).astype(F8NP),
            "wv": np.ascontiguousarray(vcols).astype(np.float16),
            "wo": np.ascontiguousarray(
                w_out[g * GDIM:(g + 1) * GDIM, :].reshape(4, 128, D)
            ).astype(np.float16),
            "bias": (b_out if g == 0 else np.zeros_like(b_out)),
        })
    return maps


def kernel(x, w_qkv, w_out, b_out):
    from concourse.bass_utils import run_bass_kernel_spmd

    if "nc" not in _CACHE:
        _CACHE["nc"] = _build()
    nc = _CACHE["nc"]
    maps = _in_maps(x, w_qkv, w_out, b_out)
    res = run_bass_kernel_spmd(nc, maps, core_ids=list(range(NCORES)))
    outs = res.results
    y = np.empty((B, N, D), dtype=np.float32)
    for b in range(B):
        y[b] = outs[2 * b]["out"] + outs[2 * b + 1]["out"]
    return y

